# revision 1
# baseline (speedup 1.0000x reference)
"""AttentionPairBias sharded across 8 NeuronCores.

Sharding (per spec hint): batch x query-rows. Core d handles batch d//4,
query rows [(d%4)*192, (d%4)*192+192). z_ij/beta_ij are split on i; k/v
are computed per-device from the (replicated) per-batch activations, so
no collectives are needed. Weights replicated.
"""

import numpy as np
import jax
import jax.numpy as jnp
from functools import partial

B, I, C_A, C_S, C_Z, H, D = 2, 768, 768, 384, 128, 16, 48
HC = H * D
EPS = 1e-5
NCORE = 8
SPLIT = 4          # i-splits per batch
IB = I // SPLIT    # 192 rows per core


def _ln(x, w=None, b=None):
    m = x.mean(-1, keepdims=True)
    v = ((x - m) ** 2).mean(-1, keepdims=True)
    y = (x - m) * jax.lax.rsqrt(v + EPS)
    if w is not None:
        y = y * w + b
    return y


def _device_fn(i0, a_full, s_full, z_loc, beta_loc, w):
    # AdaLN on the full batch (k/v need all rows)
    a = _ln(a_full)
    s = _ln(s_full, w['adaln_lns_w'], w['adaln_lns_b'])
    a = jax.nn.sigmoid(s @ w['adaln_Ws'] + w['adaln_bs']) * a + s @ w['adaln_Wnb']

    k = (a @ w['Wk']).reshape(I, H, D)
    v = (a @ w['Wv']).reshape(I, H, D)

    a_loc = jax.lax.dynamic_slice_in_dim(a, i0, IB)
    s_i_loc = jax.lax.dynamic_slice_in_dim(s_full, i0, IB)
    q = (a_loc @ w['Wq'] + w['bq']).reshape(IB, H, D)
    g = jax.nn.sigmoid(a_loc @ w['Wg']).reshape(IB, H, D)

    # pair bias for local i rows
    b_ij = _ln(z_loc, w['lnb_w'], w['lnb_b']) @ w['Wb'] + beta_loc

    scores = jnp.einsum('ihd,jhd->ijh', q, k) / (D ** 0.5) + b_ij
    A = jax.nn.softmax(scores, axis=1)

    o = jnp.einsum('ijh,jhd->ihd', A, v) * g
    out = o.reshape(IB, HC) @ w['Wo']
    out = jax.nn.sigmoid(s_i_loc @ w['Ws_out'] + w['bs_out']) * out
    return out


_pfn = jax.pmap(_device_fn, in_axes=(0, 0, 0, 0, 0, None))


def kernel(**inputs):
    inputs = {k: np.asarray(v) for k, v in inputs.items()}
    wnames = ['adaln_lns_w', 'adaln_lns_b', 'adaln_Ws', 'adaln_bs', 'adaln_Wnb',
              'Wq', 'bq', 'Wk', 'Wv', 'lnb_w', 'lnb_b', 'Wb', 'Wg', 'Wo',
              'Ws_out', 'bs_out']
    w = {n: jnp.asarray(inputs[n]) for n in wnames}

    a_i, s_i = inputs['a_i'], inputs['s_i']
    z_ij, beta_ij = inputs['z_ij'], inputs['beta_ij']

    i0 = np.array([(d % SPLIT) * IB for d in range(NCORE)], dtype=np.int32)
    bidx = [d // SPLIT for d in range(NCORE)]
    a_st = np.stack([a_i[b] for b in bidx])                       # [8,768,768]
    s_st = np.stack([s_i[b] for b in bidx])                       # [8,768,384]
    z_st = np.stack([z_ij[bidx[d], i0[d]:i0[d] + IB] for d in range(NCORE)])
    beta_st = np.stack([beta_ij[bidx[d], i0[d]:i0[d] + IB] for d in range(NCORE)])

    res = _pfn(jnp.asarray(i0), jnp.asarray(a_st), jnp.asarray(s_st),
               jnp.asarray(z_st), jnp.asarray(beta_st), w)
    res = np.asarray(res)                                         # [8,192,768]
    out = res.reshape(B, SPLIT * IB, C_A).astype(np.float32)
    return out


# revision 2
# speedup vs baseline: 1.2172x; 1.2172x over previous
"""AttentionPairBias sharded across 8 NeuronCores.

Sharding (per spec hint): batch x query-rows. Core d handles batch d//4,
query rows [(d%4)*192, (d%4)*192+192). z_ij/beta_ij are split on i
(zero-copy reshape views); k/v are computed per-device from the
(replicated) per-batch activations, so no collectives are needed.
Weights replicated.
"""

import numpy as np
import jax
import jax.numpy as jnp

B, I, C_A, C_S, C_Z, H, D = 2, 768, 768, 384, 128, 16, 48
HC = H * D
EPS = 1e-5
NCORE = 8
SPLIT = 4          # i-splits per batch
IB = I // SPLIT    # 192 rows per core


def _ln(x, w=None, b=None):
    m = x.mean(-1, keepdims=True)
    v = ((x - m) ** 2).mean(-1, keepdims=True)
    y = (x - m) * jax.lax.rsqrt(v + EPS)
    if w is not None:
        y = y * w + b
    return y


def _device_fn(i0, a_full, s_full, z_loc, beta_loc, w):
    # AdaLN on the full batch (k/v need all rows)
    a = _ln(a_full)
    s = _ln(s_full, w['adaln_lns_w'], w['adaln_lns_b'])
    a = jax.nn.sigmoid(s @ w['adaln_Ws'] + w['adaln_bs']) * a + s @ w['adaln_Wnb']

    k = (a @ w['Wk']).reshape(I, H, D)
    v = (a @ w['Wv']).reshape(I, H, D)

    a_loc = jax.lax.dynamic_slice_in_dim(a, i0, IB)
    s_i_loc = jax.lax.dynamic_slice_in_dim(s_full, i0, IB)
    q = (a_loc @ w['Wq'] + w['bq']).reshape(IB, H, D)
    g = jax.nn.sigmoid(a_loc @ w['Wg']).reshape(IB, H, D)

    # pair bias for local i rows (flat 2D layout lowers best on neuron)
    zf = z_loc.reshape(IB * I, C_Z)
    b_ij = (_ln(zf, w['lnb_w'], w['lnb_b']) @ w['Wb']).reshape(IB, I, H) + beta_loc

    scores = jnp.einsum('ihd,jhd->ijh', q, k) / (D ** 0.5) + b_ij
    A = jax.nn.softmax(scores, axis=1)

    o = jnp.einsum('ijh,jhd->ihd', A, v) * g
    out = o.reshape(IB, HC) @ w['Wo']
    out = jax.nn.sigmoid(s_i_loc @ w['Ws_out'] + w['bs_out']) * out
    return out


_pfn = jax.pmap(_device_fn, in_axes=(0, 0, 0, 0, 0, None))


def kernel(**inputs):
    inputs = {k: np.asarray(v) for k, v in inputs.items()}
    wnames = ['adaln_lns_w', 'adaln_lns_b', 'adaln_Ws', 'adaln_bs', 'adaln_Wnb',
              'Wq', 'bq', 'Wk', 'Wv', 'lnb_w', 'lnb_b', 'Wb', 'Wg', 'Wo',
              'Ws_out', 'bs_out']
    w = {n: jnp.asarray(inputs[n]) for n in wnames}

    a_i, s_i = inputs['a_i'], inputs['s_i']

    i0 = np.array([(d % SPLIT) * IB for d in range(NCORE)], dtype=np.int32)
    bidx = [d // SPLIT for d in range(NCORE)]
    # zero-copy shard views: [B, I, ...] -> [8, IB, ...]
    z_st = inputs['z_ij'].reshape(NCORE, IB, I, C_Z)
    beta_st = inputs['beta_ij'].reshape(NCORE, IB, I, H)

    devs = jax.devices()[:NCORE]
    z_sh = jax.device_put_sharded([z_st[d] for d in range(NCORE)], devs)
    beta_sh = jax.device_put_sharded([beta_st[d] for d in range(NCORE)], devs)
    a_sh = jax.device_put_sharded([a_i[b] for b in bidx], devs)
    s_sh = jax.device_put_sharded([s_i[b] for b in bidx], devs)
    i0_sh = jax.device_put_sharded(list(i0), devs)

    res = _pfn(i0_sh, a_sh, s_sh, z_sh, beta_sh, w)
    res = np.asarray(res)                                         # [8,192,768]
    out = res.reshape(B, SPLIT * IB, C_A).astype(np.float32)
    return out
